# revision 69
# baseline (speedup 1.0000x reference)
"""MLA Trainium2 kernel v10, 8-core SPMD.  (674us baseline -> 582us)

Sharding: batch (2) x head-groups (4 groups of 4 heads) -> 8 cores; host
sums the 4 partial outputs per batch.

Key changes vs the v2 baseline:
- rms paths use scalar AF.Rsqrt (raw InstActivation — same ACT table set
  as Square/Copy), killing the slow Vector RECIPROCALs that stalled the
  PE at the A->B boundary; softmax den reciprocals run on the [1,512]
  rows so no PSUM bank is held during them.
- Phase order: per-chunk [fp8 q-ssq slice + kv latent] (loop1, keeps the
  PE dense through the bandwidth-bound prologue) -> gather fire ->
  folded-q/krope chunks (loop2) -> B (k/v up-proj) -> alpha +
  q-normalize on GpSimd (hidden under B) -> attention.
- All weights/activations host-packed in exact SBUF layouts: contiguous
  DMAs are ~16x fewer descriptors (descriptor-gen was serializing the
  weight stream); x fp8 is chunk-0-only from host, chunks 1-3 are cast
  on-device (fp8 DR layout is partition-identical to xT).
- Attention processes head PAIRS: the two K=64 rope score matmuls
  row-tile into the two PE array halves (krope2 replicated per half,
  tile_position), and the two PD denominator chains col-tile into one
  PSUM bank (1-wide outputs at partitions 0/64).
- emit (wo) lags its chunk's last finalize by one pair-unit so oh
  writes never stall the wo stream.
"""
import os
import sys
import numpy as np
from contextlib import ExitStack

sys.path.insert(0, "/opt/trn_rl_repo")

import ml_dtypes
import concourse.bass as bass
import concourse.mybir as mybir
import concourse.tile as tile
from concourse.bass_utils import run_bass_kernel_spmd

F32 = mybir.dt.float32
F32R = mybir.dt.float32r
BF16 = mybir.dt.bfloat16
FP8 = mybir.dt.float8e4
AF = mybir.ActivationFunctionType
DR = mybir.MatmulPerfMode.DoubleRow
NPBF = ml_dtypes.bfloat16
NPF8 = ml_dtypes.float8_e4m3

B, S, D = 2, 2048, 2048
H, NOPE, RD, VD = 16, 128, 64, 128
QR, KVR, EPS = 1536, 512, 1e-6
HPC = 4
SCALE = 1.0 / np.sqrt(NOPE + RD)
NEG = -1e30
QSL = QR // 4          # 384 q-latent dims per core (norm slice)
KSL = KVR // 4         # 128 kv-latent dims per core
X8S = 4.0              # fp8 scale for x
W8S = 512.0            # fp8 scale for wq_down norm slice
SS_SCALE = 1.0 / (QR * (X8S * W8S) ** 2)

_CACHED = {}


def _act_rsqrt(nc, out, in_, bias, scale):
    """ACTIVATE Rsqrt via direct InstActivation (the bass-level wrapper
    refuses Rsqrt on accuracy grounds; table precision is plenty for a
    bf16 pipeline and it keeps phase A/B on one table set)."""
    eng = nc.scalar
    inputs = [eng.lower_ap(in_), eng.lower_ap(bias),
              mybir.ImmediateValue(dtype=mybir.dt.float32, value=float(scale)),
              mybir.ImmediateValue(dtype=mybir.dt.float32, value=0.0)]
    return eng.add_instruction(
        mybir.InstActivation(
            name=nc.get_next_instruction_name(),
            func=mybir.ActivationFunctionType.Rsqrt,
            ins=inputs,
            outs=[eng.lower_ap(out)],
        )
    )


def _split_waits(nc, max_waits=1):
    ctr = 0
    for f in nc.m.functions:
        for bb in f.blocks:
            insts = list(bb.instructions)
            out = []
            changed = False
            for inst in insts:
                si = inst.sync_info
                waits = list(si.on_wait) if si is not None else []
                if len(waits) > max_waits:
                    changed = True
                    head, rest = waits[:max_waits], waits[max_waits:]
                    while rest:
                        ctr += 1
                        nop = mybir.InstNoOp(name=f"WS-{ctr}")
                        nop.engine = inst.engine
                        nop.sync_info = mybir.SyncInfo(on_wait=head, on_update=[])
                        out.append(nop)
                        head, rest = rest[:max_waits], rest[max_waits:]
                    inst.sync_info = mybir.SyncInfo(
                        on_wait=head, on_update=list(si.on_update))
                out.append(inst)
            if changed:
                bb.instructions = out


def _build_nc():
    nc = bass.Bass("TRN2", target_bir_lowering=False, debug=False, num_devices=8)

    def din(name, shape, dt):
        return nc.dram_tensor(name, list(shape), dt, kind="ExternalInput").ap()

    # all tensors are host-packed in their exact SBUF layouts (contiguous
    # per chunk) so every DMA needs ~128 descriptors instead of ~2048 —
    # descriptor generation on the issuing engine was serializing the
    # whole weight stream
    xT = din("xT", [4, 128, 16, 512], BF16)       # x[b].T chunk-major
    x8 = din("x8", [128, 8, 2, 512], FP8)         # fp8 DR layout, chunk 0 only
    wqn = din("wqn", [128, 16, HPC * NOPE], BF16)  # folded q_nope
    wqr = din("wqr", [128, 16, HPC * RD], BF16)   # folded q_rope
    wnq = din("wnq", [128, 24, 2, 128], FP8)      # wq_down norm slice, DR fp8
    # wnq index: [p, j*3+m, i, c] = wqd[256j+128i+p, 128m+c] * W8S
    wkv = din("wkv", [128, 16, KVR], BF16)        # wkv_down (full)
    wkuk = din("wkuk", [128, 4, HPC * NOPE], BF16)  # kv->k_nope up
    wkuv = din("wkuv", [128, 4, HPC * VD], BF16)  # kv->v up (gamma folded)
    wkr = din("wkr", [128, 16, RD], BF16)
    wo = din("wo", [128, HPC, D], BF16)
    cosq = din("cosq", [128, S], BF16)            # 2-head-stacked rope tables
    sinq = din("sinq", [128, S], BF16)
    cosk = din("cosk", [RD, S], BF16)
    sink = din("sink", [RD, S], BF16)
    perm128 = din("perm128", [128, 128], BF16)
    perm64 = din("perm64", [RD, RD], BF16)
    id128 = din("id128", [128, 128], BF16)
    trimaskb = din("trimaskb", [128, 128], BF16)
    ones_col = din("ones_col", [128, 1], F32R)
    ones_colb = din("ones_colb", [128, 1], BF16)
    ones_row = din("ones_row", [1, 128], F32R)
    eps_col = din("eps_col", [128, 1], F32)
    trimask = din("trimask", [128, 128], F32)
    trimask512 = din("trimask512", [128, 512], F32)

    outT = nc.dram_tensor("outT", [16, 4, 128, 512], BF16,
                          kind="ExternalOutput").ap()

    # collective plumbing (DRAM). ONE AllGather carries the q-sumsq
    # partials as bf16 hi/lo row pairs (exact to ~1.5e-5 after the local
    # re-sum).
    ss_sh = nc.dram_tensor("ss_sh", [4, 2, S], BF16, kind="Internal").ap()
    groups = [[0, 1, 2, 3], [4, 5, 6, 7]]



    with tile.TileContext(nc) as tc, ExitStack() as top:
        dpool = top.enter_context(tc.tile_pool(name="dram", bufs=1, space="DRAM"))
        ss_in = dpool.tile([2, S], BF16, tag="ssi", name="ss_in")
        rcp_d = dpool.tile([1, S], F32, tag="rcd", name="rcp_d")

        consts = top.enter_context(tc.tile_pool(name="consts", bufs=1))
        ones_c = consts.tile([128, 1], F32R, tag="oc", name="ones_c")
        ones_cb = consts.tile([128, 1], BF16, tag="ocb", name="ones_cb")
        ones_r = consts.tile([1, 128], F32R, tag="orr", name="ones_r")
        eps_c = consts.tile([128, 1], F32, tag="eps", name="eps_c")
        tri_sb = consts.tile([128, 128], F32, tag="tri", name="tri_sb")
        perm_sb = consts.tile([128, 128], BF16, tag="pm", name="perm_sb")
        perm64_sb = consts.tile([RD, RD], BF16, tag="pm64", name="perm64_sb")

        # long-lived activation tiles (qn/qr normalized in place later)
        acts = top.enter_context(tc.tile_pool(name="acts", bufs=1))
        qn = [acts.tile([128, S], BF16, tag=f"qnr{h}", name=f"qn_raw{h}")
              for h in range(HPC)]
        qr_rot = [acts.tile([128, S], BF16, tag=f"qrr{p}", name=f"qr_rot{p}")
                  for p in range(2)]
        # krope duplicated in both partition halves so per-head rope score
        # matmuls can row-tile into array halves and run concurrently
        krope2 = acts.tile([128, S], BF16, tag="kro", name="krope2")
        kvn = acts.tile([128, 4, S], BF16, tag="kvn", name="kvn")
        rsq_kv = acts.tile([1, S], F32, tag="rqk", name="rsq_kv")
        rsq_kvr = acts.tile([1, S], F32R, tag="rqkr", name="rsq_kvr")

        # phase-B weights, prefetched during phase A
        p_bw = top.enter_context(tc.tile_pool(name="bw", bufs=1))
        wkuk_sb = p_bw.tile([128, 4, HPC * NOPE], BF16, tag="wkuk",
                            name="wkuk_sb")
        wkuv_sb = p_bw.tile([128, 4, HPC * VD], BF16, tag="wkuv",
                            name="wkuv_sb")

        # ---------------- Phase A: norm slices, then fused x-pass ---------
        with ExitStack() as sA:
            psum = sA.enter_context(tc.tile_pool(name="aP", bufs=1, space="PSUM"))
            p_w = sA.enter_context(tc.tile_pool(name="aW", bufs=1))
            p_x = sA.enter_context(tc.tile_pool(name="aX", bufs=1))
            p_t = sA.enter_context(tc.tile_pool(name="aT", bufs=1))

            def load_xt(n):
                t = p_x.tile([128, 16, 512], BF16, tag="xt", bufs=4,
                             name=f"ax{n}")
                nc.sync.dma_start(t[:], xT[n])
                return t

            with ExitStack() as sN:
                p_n = sN.enter_context(tc.tile_pool(name="aN", bufs=1))
                wnq_sb = p_n.tile([128, 24, 2, 128], FP8, tag="wnq",
                                  name="wnq_sb")
                nc.sync.dma_start(wnq_sb[:], wnq[:])
                nc.sync.dma_start(ones_c[:], ones_col[:])
                nc.sync.dma_start(ones_cb[:], ones_colb[:])
                nc.sync.dma_start(ones_r[:], ones_row[:])
                nc.sync.dma_start(eps_c[:], eps_col[:])
                # critical x-pass inputs stream during the ssq loop; the
                # rest of the weight set is deferred until after it so
                # these transfers aren't bandwidth-starved
                wkv_sb = p_w.tile([128, 16, KVR], BF16, tag="wkv",
                                  name="wkv_sb")
                nc.sync.dma_start(wkv_sb[:], wkv[:])
                xts = {n: load_xt(n) for n in range(4)}
                hi_q = p_n.tile([1, S], BF16, tag="hiq", name="hi_q")
                lo_q = p_n.tile([1, S], BF16, tag="loq", name="lo_q")
                # loop1: per chunk, fp8 q-ssq slice THEN the kv latent —
                # ~21us of PE work per chunk covers the xt DMA cadence so
                # the bandwidth-bound prologue never exposes a gap.
                # Chunk 0's fp8 pack comes from the host (its xt may still
                # be in flight); later chunks are cast on-device.
                for n in range(4):
                    sl = slice(n * 512, n * 512 + 512)
                    x8t = p_n.tile([128, 8, 2, 512], FP8, tag="x8", bufs=2,
                                   name=f"a1x8{n}")
                    if n == 0:
                        nc.gpsimd.dma_start(x8t[:], x8[:])
                    else:
                        for jj in range(16):
                            src = xts[n][:, jj, :]
                            dst = x8t[:, jj // 2, jj % 2, :]
                            if jj % 2 == 0:
                                nc.scalar.mul(dst, src, X8S)
                            else:
                                nc.vector.tensor_scalar_mul(dst, src, X8S)
                    pss = psum.tile([1, 512], F32, tag="ssq", name=f"pss{n}")
                    for m in range(3):
                        pq = psum.tile([128, 512], F32, tag="mm", bufs=4,
                                       name=f"nq{n}_{m}")
                        for j in range(8):
                            nc.tensor.matmul(
                                pq[:], wnq_sb[:, j * 3 + m, :, :],
                                x8t[:, j, :, :], start=(j == 0),
                                stop=(j == 7), perf_mode=DR,
                                skip_group_check=True)
                        sq = p_n.tile([128, 512], F32R, tag="sq", bufs=2,
                                      name=f"sq{n}_{m}")
                        nc.scalar.activation(sq[:], pq[:], AF.Square)
                        nc.tensor.matmul(pss[:], ones_c[:], sq[:],
                                         start=(m == 0), stop=(m == 2),
                                         skip_group_check=True)
                    nc.vector.tensor_copy(hi_q[:, sl], pss[:])
                    nc.vector.tensor_sub(lo_q[:, sl], pss[:], hi_q[:, sl])
                    # kv_c (full latent) + sumsq; paired chains hide LDW
                    pskv = psum.tile([1, 512], F32, tag="ssq",
                                     name=f"pskv{n}")
                    sqvs = []
                    pk = {}
                    for m in range(4):
                        pk[m] = psum.tile([128, 512], F32, tag="mm", bufs=4,
                                          name=f"kv{n}_{m}")
                    for k in range(16):
                        for m in range(4):
                            nc.tensor.matmul(
                                pk[m][:], wkv_sb[:, k, m * 128:(m + 1) * 128],
                                xts[n][:, k, :], start=(k == 0),
                                stop=(k == 15), skip_group_check=True)
                    for m in range(4):
                        if m % 2 == 0:
                            nc.scalar.copy(kvn[:, m, sl], pk[m][:])
                        else:
                            nc.vector.tensor_copy(kvn[:, m, sl], pk[m][:])
                        sqv = p_t.tile([128, 512], F32R, tag="sqv", bufs=4,
                                       name=f"sqv{n}_{m}")
                        nc.scalar.activation(sqv[:], pk[m][:], AF.Square)
                        sqvs.append(sqv)
                    for m, sqv in enumerate(sqvs):
                        nc.tensor.matmul(pskv[:], ones_c[:], sqv[:],
                                         start=(m == 0), stop=(m == 3),
                                         skip_group_check=True)
                    _act_rsqrt(nc, rsq_kv[:, sl], pskv[:],
                               bias=eps_c[0:1, :], scale=1.0 / KVR)
                    nc.vector.tensor_copy(rsq_kvr[:, sl], rsq_kv[:, sl])
                # tiny q-sumsq gather (the only collective)
                nc.gpsimd.dma_start(ss_in[0:1, :], hi_q[:])
                nc.gpsimd.dma_start(ss_in[1:2, :], lo_q[:])
                nc.gpsimd.collective_compute(
                    "AllGather", mybir.AluOpType.bypass,
                    replica_groups=groups, ins=[ss_in[:]], outs=[ss_sh])

            # fused x-pass: folded q, krope. This weight set is deferred
            # until after the ssq loop so its ~7MB doesn't starve the
            # critical wkv/xt transfers above (own pool: reuses the SBUF
            # the ssq scope just freed).
            p_w2 = sA.enter_context(tc.tile_pool(name="aW2", bufs=1))
            wqn_sb = p_w2.tile([128, 16, HPC * NOPE], BF16, tag="wqn",
                              name="wqn_sb")
            nc.sync.dma_start(wqn_sb[:], wqn[:])
            wqr_sb = p_w2.tile([128, 16, HPC * RD], BF16, tag="wqr",
                              name="wqr_sb")
            nc.sync.dma_start(wqr_sb[:], wqr[:])
            wkr_sb = p_w2.tile([128, 16, RD], BF16, tag="wkr", name="wkr_sb")
            nc.sync.dma_start(wkr_sb[:], wkr[:])
            cq = p_w2.tile([128, S], BF16, tag="cq", name="cq")
            nc.sync.dma_start(cq[:], cosq[:])
            sq_ = p_w2.tile([128, S], BF16, tag="sqt", name="sqt")
            nc.sync.dma_start(sq_[:], sinq[:])
            nc.sync.dma_start(tri_sb[:], trimask[:])
            nc.sync.dma_start(perm_sb[:], perm128[:])
            nc.sync.dma_start(perm64_sb[:], perm64[:])
            # prefetch phase-B weights (used ~100us later)
            nc.sync.dma_start(wkuk_sb[:], wkuk[:])
            nc.sync.dma_start(wkuv_sb[:], wkuv[:])

            for n in range(4):
                sl = slice(n * 512, n * 512 + 512)
                xt = xts[n]
                # folded q_nope (4-way interleaved chains)
                ps = {}
                for mh in range(HPC):
                    ps[mh] = psum.tile([128, 512], F32, tag="mm", bufs=4,
                                       name=f"qn{n}_{mh}")
                for k in range(16):
                    for mh in range(HPC):
                        nc.tensor.matmul(
                            ps[mh][:], wqn_sb[:, k, mh * 128:(mh + 1) * 128],
                            xt[:, k, :], start=(k == 0), stop=(k == 15),
                            skip_group_check=True)
                for mh in range(HPC):
                    if mh % 2 == 0:
                        nc.scalar.copy(qn[mh][:, sl], ps[mh][:])
                    else:
                        nc.vector.tensor_copy(qn[mh][:, sl], ps[mh][:])
                # folded q_rope (paired) + rope (pre-alpha)
                ps = {}
                for mr in range(2):
                    ps[mr] = psum.tile([128, 512], F32, tag="mm", bufs=4,
                                       name=f"qr{n}_{mr}")
                for k in range(16):
                    for mr in range(2):
                        nc.tensor.matmul(ps[mr][:],
                                         wqr_sb[:, k, mr * 128:(mr + 1) * 128],
                                         xt[:, k, :], start=(k == 0),
                                         stop=(k == 15), skip_group_check=True)
                for mr in range(2):
                    psw = psum.tile([128, 512], F32, tag="sw", bufs=1,
                                    name=f"sw{mr}_{n}")
                    raw = p_t.tile([128, 512], BF16, tag="qraw", bufs=2,
                                   name=f"qraw{mr}_{n}")
                    nc.scalar.copy(raw[:], ps[mr][:])
                    nc.tensor.matmul(psw[:], perm_sb[:], raw[:],
                                     start=True, stop=True,
                                     skip_group_check=True)
                    t1 = p_t.tile([128, 512], F32, tag="t1", bufs=2,
                                  name=f"t1{mr}_{n}")
                    nc.vector.tensor_mul(t1[:], ps[mr][:], cq[:, sl])
                    t2 = p_t.tile([128, 512], F32, tag="t2", bufs=2,
                                  name=f"t2{mr}_{n}")
                    nc.vector.tensor_mul(t2[:], psw[:], sq_[:, sl])
                    nc.vector.tensor_add(qr_rot[mr][:, sl], t1[:], t2[:])
                # krope + rope
                psk = psum.tile([RD, 512], F32, tag="mmk", bufs=1,
                                name=f"kr{n}")
                for k in range(16):
                    nc.tensor.matmul(psk[:], wkr_sb[:, k, :], xt[:, k, :],
                                     start=(k == 0), stop=(k == 15),
                                     skip_group_check=True)
                raw = p_t.tile([RD, 512], BF16, tag="kraw", bufs=2,
                               name=f"kraw{n}")
                nc.scalar.copy(raw[:], psk[:])
                psw = psum.tile([RD, 512], F32, tag="sw", bufs=1,
                                name=f"swk{n}")
                nc.tensor.matmul(psw[:], perm64_sb[:], raw[:],
                                 start=True, stop=True, skip_group_check=True)
                t1 = p_t.tile([128, 512], F32, tag="t1", bufs=2,
                              name=f"kt1{n}")
                nc.vector.tensor_mul(t1[0:RD, :], psk[:], cq[0:RD, sl])
                t2 = p_t.tile([128, 512], F32, tag="t2", bufs=2,
                              name=f"kt2{n}")
                nc.vector.tensor_mul(t2[0:RD, :], psw[:], sq_[0:RD, sl])
                nc.vector.tensor_add(krope2[0:RD, sl], t1[0:RD, :],
                                     t2[0:RD, :])

        # late long-lived tiles (allocated after phase A frees its pools)
        p_at = top.enter_context(tc.tile_pool(name="att", bufs=1))
        p_late = top.enter_context(tc.tile_pool(name="late", bufs=1))
        knope = [p_late.tile([128, S], BF16, tag=f"kn{h}", name=f"knope{h}")
                 for h in range(HPC)]
        v_sb = p_late.tile([128, 16, HPC * VD], BF16, tag="vsb", name="v_sb")
        oh = [p_late.tile([128, S], BF16, tag=f"oh{h}", name=f"oh{h}")
              for h in range(HPC)]

        # ---------------- Phase B: k_nope / v up-projections --------------
        # (also computes alpha from the gather and normalizes q on GpSimd,
        # all hidden under B's matmul stream)
        with ExitStack() as sb_:
            p_b = sb_.enter_context(tc.tile_pool(name="b_w", bufs=1))
            psum = sb_.enter_context(tc.tile_pool(name="b_p", bufs=1, space="PSUM"))
            beta_bc = p_b.tile([128, S], BF16, tag="bbc", name="beta_bc")
            beta_t = p_b.tile([128, 16], F32, tag="btr", name="beta_t")
            # replicate krope into the upper partition half (for row-tiled
            # rope score matmuls in phase D)
            nc.sync.dma_start(krope2[RD:2 * RD, :], krope2[0:RD, :])
            for n in range(4):
                sl = slice(n * 512, n * 512 + 512)
                pb_ = psum.tile([128, 512], F32, tag="pbcb", bufs=2,
                                name=f"pbB{n}")
                nc.tensor.matmul(pb_[:], ones_r[:], rsq_kvr[:, sl], start=True,
                                 stop=True, skip_group_check=True)
                if n % 2 == 0:
                    nc.scalar.copy(beta_bc[:, sl], pb_[:])
                else:
                    nc.vector.tensor_copy(beta_bc[:, sl], pb_[:])
            nc.sync.dma_start(rcp_d[:], rsq_kv[:])
            nc.sync.dma_start(
                beta_t[:], rcp_d[:].rearrange("o (n p) -> p (o n)", p=128))

            # alpha inputs (gather landed ~50us ago)
            alpha_bc = p_b.tile([128, S], BF16, tag="abc", name="alpha_bc")
            ssgq = p_b.tile([8, S], BF16, tag="ssgq", name="ssgq")
            for g in range(4):
                nc.gpsimd.dma_start(ssgq[2 * g:2 * g + 2, :], ss_sh[g])
            rsq_q = p_b.tile([1, S], F32, tag="rqq", name="rsq_q")
            rsq_qr = p_b.tile([1, S], F32R, tag="rqqr", name="rsq_qr")

            for mh in range(HPC):
                ps = {}
                for c in range(4):
                    ps[c] = psum.tile([128, 512], F32, tag="mmb",
                                      bufs=4, name=f"knp{mh}_{c}")
                for k in range(4):
                    for c in range(4):
                        sl = slice(c * 512, c * 512 + 512)
                        nc.tensor.matmul(
                            ps[c][:], wkuk_sb[:, k, mh * 128:(mh + 1) * 128],
                            kvn[:, k, sl], start=(k == 0), stop=(k == 3),
                            skip_group_check=True)
                for c in range(4):
                    sl = slice(c * 512, c * 512 + 512)
                    nc.vector.tensor_mul(knope[mh][:, sl], ps[c][:],
                                         beta_bc[:, sl])

            # alpha: small matmuls between the two B sections so the PE
            # stream isn't gated on them and alpha_bc is ready before D
            for n in range(4):
                sl = slice(n * 512, n * 512 + 512)
                pss2 = psum.tile([1, 512], F32, tag="ssq2", bufs=2,
                                 name=f"ssq2_{n}")
                nc.tensor.matmul(pss2[:], ones_cb[0:8, :], ssgq[:, sl],
                                 start=True, stop=True, skip_group_check=True)
                _act_rsqrt(nc, rsq_q[:, sl], pss2[:],
                           bias=eps_c[0:1, :], scale=SS_SCALE)
                nc.vector.tensor_copy(rsq_qr[:, sl], rsq_q[:, sl])
            for n in range(4):
                sl = slice(n * 512, n * 512 + 512)
                pa = psum.tile([128, 512], F32, tag="pbcb", bufs=2,
                               name=f"pa{n}")
                nc.tensor.matmul(pa[:], ones_r[:], rsq_qr[:, sl], start=True,
                                 stop=True, skip_group_check=True)
                nc.scalar.copy(alpha_bc[:, sl], pa[:])

            for tp in range(4):
                psv = {}
                for ti in range(4):
                    t = 4 * tp + ti
                    psv[ti] = psum.tile([128, 512], F32, tag="mmb", bufs=4,
                                        name=f"v{t}")
                for k in range(4):
                    for ti in range(4):
                        t = 4 * tp + ti
                        nc.tensor.matmul(psv[ti][:],
                                         kvn[:, k, t * 128:(t + 1) * 128],
                                         wkuv_sb[:, k, :], start=(k == 0),
                                         stop=(k == 3), skip_group_check=True)
                for ti in range(4):
                    t = 4 * tp + ti
                    if ti % 2 == 0:
                        nc.vector.tensor_scalar_mul(v_sb[:, t, :], psv[ti][:],
                                                    beta_t[:, t:t + 1])
                    else:
                        nc.scalar.activation(v_sb[:, t, :], psv[ti][:],
                                             AF.Copy,
                                             scale=beta_t[:, t:t + 1])

            # q-normalize on GpSimd (idle engine; overlaps the v section)
            for h in range(HPC):
                nc.gpsimd.tensor_mul(qn[h][:], qn[h][:], alpha_bc[:])
            for p in range(2):
                nc.gpsimd.tensor_mul(qr_rot[p][:], qr_rot[p][:], alpha_bc[:])

        # ---------------- Phase D: attention ------------------------------
        p_wo = top.enter_context(tc.tile_pool(name="pwo", bufs=1))
        wo_sb = p_wo.tile([128, HPC, D], BF16, tag="wo", name="wo_sb")
        nc.sync.dma_start(wo_sb[:], wo[:])
        with ExitStack() as s3:
            psum = s3.enter_context(tc.tile_pool(name="d_p", bufs=1, space="PSUM"))

            def unit_scores(p, c):
                """j-loop for a head PAIR (2p, 2p+1) on one query-chunk.

                The two heads' rope matmuls (K=64) run concurrently in the
                two row halves of the PE array (krope2 holds the shared
                rope key in both partition halves; qr_rot[p] already has
                the even head in partitions 0:64 and the odd in 64:128).
                The two PD denominator chains col-tile into one PSUM bank
                (1-wide outputs at base partitions 0 and 64)."""
                he, ho = 2 * p, 2 * p + 1
                s0 = c * 512
                po_e = psum.tile([128, 512], F32, tag="po", bufs=4,
                                 name=f"poe{p}_{c}")
                po_o = psum.tile([128, 512], F32, tag="po", bufs=4,
                                 name=f"poo{p}_{c}")
                pdp = psum.tile([128, 512], F32, tag="pd", bufs=1,
                                name=f"pd{p}_{c}")
                jmax = 4 * c + 3
                for j in range(jmax + 1):
                    off = max(0, 128 * (j - 4 * c))
                    npx = 512 - off
                    sc_e = psum.tile([128, 512], F32, tag="sc", bufs=3,
                                     name=f"sce{p}_{c}_{j}")
                    sc_o = psum.tile([128, 512], F32, tag="sc", bufs=3,
                                     name=f"sco{p}_{c}_{j}")
                    jb = slice(j * 128, (j + 1) * 128)
                    qs = slice(s0 + off, s0 + 512)
                    nc.tensor.matmul(sc_e[:, off:], knope[he][:, jb],
                                     qn[he][:, qs], start=True, stop=False,
                                     skip_group_check=True)
                    nc.tensor.matmul(sc_o[:, off:], knope[ho][:, jb],
                                     qn[ho][:, qs], start=True, stop=False,
                                     skip_group_check=True)
                    nc.tensor.matmul(sc_e[:, off:], krope2[0:RD, jb],
                                     qr_rot[p][0:RD, qs],
                                     start=False, stop=True,
                                     tile_position=(0, 0),
                                     skip_group_check=True)
                    nc.tensor.matmul(sc_o[:, off:], krope2[RD:2 * RD, jb],
                                     qr_rot[p][RD:2 * RD, qs],
                                     start=False, stop=True,
                                     tile_position=(RD, 0),
                                     skip_group_check=True)
                    if j >= 4 * c:
                        nc.vector.tensor_add(sc_e[:, off:off + 128],
                                             sc_e[:, off:off + 128],
                                             tri_sb[:])
                        nc.vector.tensor_add(sc_o[:, off:off + 128],
                                             sc_o[:, off:off + 128],
                                             tri_sb[:])
                    pr_e = p_at.tile([128, 512], BF16, tag="pr", bufs=10,
                                     name=f"pre{p}_{c}_{j}")
                    nc.scalar.activation(pr_e[:, :npx], sc_e[:, off:], AF.Exp)
                    pr_o = p_at.tile([128, 512], BF16, tag="pr", bufs=10,
                                     name=f"pro{p}_{c}_{j}")
                    nc.scalar.activation(pr_o[:, :npx], sc_o[:, off:], AF.Exp)
                    nc.tensor.matmul(po_e[:, off:],
                                     v_sb[:, j, he * VD:(he + 1) * VD],
                                     pr_e[:, :npx], start=(j == 0),
                                     stop=(j == jmax), skip_group_check=True)
                    nc.tensor.matmul(po_o[:, off:],
                                     v_sb[:, j, ho * VD:(ho + 1) * VD],
                                     pr_o[:, :npx], start=(j == 0),
                                     stop=(j == jmax), skip_group_check=True)
                    nc.tensor.matmul(pdp[0:1, off:], ones_cb[:],
                                     pr_e[:, :npx], start=(j == 0),
                                     stop=(j == jmax),
                                     tile_position=(0, 0),
                                     skip_group_check=True)
                    nc.tensor.matmul(pdp[RD:RD + 1, off:], ones_cb[:],
                                     pr_o[:, :npx], start=(j == 0),
                                     stop=(j == jmax),
                                     tile_position=(0, RD),
                                     skip_group_check=True)
                # den rows drained to SBUF here (fast scalar copies) so
                # the shared PD PSUM bank frees before the next pair's
                # first PD matmul — otherwise that matmul waits on copies
                # queued behind the next unit's whole exp stream
                dens = []
                for h, drow in ((2 * p, 0), (2 * p + 1, RD)):
                    den = p_at.tile([1, 512], F32R, tag="den", bufs=4,
                                    name=f"den{h}_{c}")
                    nc.scalar.copy(den[:], pdp[drow:drow + 1, :])
                    dens.append(den)
                return (p, c, po_e, po_o, dens)

            def unit_finalize(u):
                """Reciprocal runs on the [1,512] den row (same DVE cost,
                but no PSUM bank is held during it); the broadcast matmul
                then carries the already-inverted row."""
                p, c, po_e, po_o, dens = u
                s0 = c * 512
                for h, po, den in ((2 * p, po_e, dens[0]),
                                   (2 * p + 1, po_o, dens[1])):
                    rcd = p_at.tile([1, 512], F32R, tag="rcd", bufs=2,
                                    name=f"rcd{h}_{c}")
                    with nc.allow_low_precision(reason="f32r recip row"):
                        nc.vector.reciprocal(rcd[:], den[:])
                    pbc_ = psum.tile([128, 512], F32, tag="sc", bufs=3,
                                     name=f"dbc{h}_{c}")
                    nc.tensor.matmul(pbc_[:], ones_r[:], rcd[:], start=True,
                                     stop=True, skip_group_check=True)
                    rcp = p_at.tile([128, 512], F32, tag="rcp", bufs=2,
                                    name=f"rcp{h}_{c}")
                    nc.vector.tensor_copy(rcp[:], pbc_[:])
                    nc.vector.tensor_mul(oh[h][:, s0:s0 + 512], po[:],
                                         rcp[:])

            def emit_E(c):
                s0 = c * 512
                for mo in range(0, 16, 2):
                    ps = {}
                    for d_ in range(2):
                        ps[d_] = psum.tile([128, 512], F32, tag="sc", bufs=3,
                                           name=f"o{mo + d_}_{c}")
                    for k in range(HPC):
                        for d_ in range(2):
                            mm = mo + d_
                            nc.tensor.matmul(
                                ps[d_][:],
                                wo_sb[:, k, mm * 128:(mm + 1) * 128],
                                oh[k][:, s0:s0 + 512], start=(k == 0),
                                stop=(k == HPC - 1), skip_group_check=True)
                    for d_ in range(2):
                        mm = mo + d_
                        fin = p_at.tile([128, 512], BF16, tag="fin", bufs=4,
                                        name=f"fin{mm}_{c}")
                        if (mm + c) % 2 == 0:
                            nc.scalar.copy(fin[:], ps[d_][:])
                        else:
                            nc.vector.tensor_copy(fin[:], ps[d_][:])
                        nc.sync.dma_start(outT[mm, c], fin[:])

            # finalize lags scores by one pair-unit; emit lags its chunk's
            # last finalize by another unit so the oh writes (gated by the
            # den reciprocals) never stall the wo matmul stream
            pending = None
            emit_c = None
            for c in range(4):
                for p in range(2):
                    u = unit_scores(p, c)
                    if pending is not None:
                        unit_finalize(pending)
                        if emit_c is not None:
                            emit_E(emit_c)
                            emit_c = None
                        if pending[0] == 1:
                            emit_c = pending[1]
                    pending = u
            unit_finalize(pending)
            if emit_c is not None:
                emit_E(emit_c)
            emit_E(3)

    _split_waits(nc, max_waits=1)
    return nc


def _host_inputs(inputs):
    x = np.asarray(inputs["x"], np.float32)
    wq_down = np.asarray(inputs["wq_down"], np.float64)
    q_norm_w = np.asarray(inputs["q_norm_w"], np.float64)
    wq_up = np.asarray(inputs["wq_up"], np.float64)
    wq_rope = np.asarray(inputs["wq_rope"], np.float64)
    wkv_down = np.asarray(inputs["wkv_down"], np.float32)
    kv_norm_w = np.asarray(inputs["kv_norm_w"], np.float64)
    wkv_up = np.asarray(inputs["wkv_up"], np.float64)
    wk_rope = np.asarray(inputs["wk_rope"], np.float32)
    wo = np.asarray(inputs["wo"], np.float32)

    # folded q weights
    Wqn = (wq_down @ (wq_up.reshape(QR, H, NOPE) *
                      q_norm_w[:, None, None] * SCALE).reshape(QR, H * NOPE)
           ).astype(np.float32)                                    # [D, H*NOPE]
    Wqr = (wq_down @ (wq_rope.reshape(QR, H, RD) *
                      q_norm_w[:, None, None] * SCALE).reshape(QR, H * RD)
           ).astype(np.float32)                                    # [D, H*RD]
    wku = (wkv_up * kv_norm_w[:, None]).astype(np.float32).reshape(
        KVR, H, NOPE + VD)

    # rope tables ([dim, s]; o = z*C + swap(z)*Ssg)
    pos = np.arange(S, dtype=np.float64)
    invf = 1.0 / (10000.0 ** (np.arange(0, RD, 2, dtype=np.float64) / RD))
    ang = invf[:, None] * pos[None, :]
    C64 = np.repeat(np.cos(ang), 2, axis=0).astype(np.float32)
    Ssg = np.empty((RD, S), np.float32)
    Ssg[0::2] = -np.sin(ang)
    Ssg[1::2] = np.sin(ang)

    perm = np.zeros((128, 128), np.float32)
    idx = np.arange(128)
    perm[idx, idx ^ 1] = 1.0

    t_i = np.arange(128)[:, None]
    s_i = np.arange(128)[None, :]
    trimask = np.where(s_i >= t_i, 0.0, NEG).astype(np.float32)
    trimask512 = np.zeros((128, 512), np.float32)
    trimask512[:, :128] = trimask

    def dr_pack(a, scale):
        # [D, 512] f32 -> [128, D//256, 2, 512] fp8 with d = 256j+128i+p
        Dd, M = a.shape
        return np.ascontiguousarray(
            (a * scale).reshape(Dd // 256, 2, 128, M).transpose(2, 0, 1, 3)
        ).astype(NPF8)

    def pk(a):
        # [D, M] -> [128, D//128, M] (SBUF weight layout, contiguous)
        Dd, M = a.shape
        return np.ascontiguousarray(
            np.asarray(a, np.float32).reshape(Dd // 128, 128, M)
            .transpose(1, 0, 2)).astype(NPBF)

    shared = {
        "wkr": pk(wk_rope),
        "cosq": np.vstack([C64, C64]).astype(NPBF),
        "sinq": np.vstack([Ssg, Ssg]).astype(NPBF),
        "cosk": C64.astype(NPBF), "sink": Ssg.astype(NPBF),
        "perm128": perm.astype(NPBF),
        "perm64": np.ascontiguousarray(perm[:RD, :RD]).astype(NPBF),
        "id128": np.eye(128, dtype=np.float32).astype(NPBF),
        "trimaskb": trimask.astype(NPBF),
        "ones_col": np.ones((128, 1), np.float32),
        "ones_colb": np.ones((128, 1), np.float32).astype(NPBF),
        "ones_row": np.ones((1, 128), np.float32),
        "eps_col": np.full((128, 1), EPS, np.float32),
        "trimask": trimask,
        "trimask512": trimask512,
    }
    in_maps = []
    for core in range(8):
        b, g = divmod(core, HPC)
        hs = slice(g * HPC, (g + 1) * HPC)
        m = dict(shared)
        xb = x[b]
        xbT = np.ascontiguousarray(xb.T)
        # [D, S] -> [4, 128, 16, 512] chunk-major
        m["xT"] = np.ascontiguousarray(
            xbT.reshape(16, 128, 4, 512).transpose(2, 1, 0, 3)).astype(NPBF)
        m["x8"] = dr_pack(xbT[:, :512], X8S)
        m["wqn"] = pk(
            Wqn.reshape(D, H, NOPE)[:, hs].reshape(D, HPC * NOPE))
        m["wqr"] = pk(
            Wqr.reshape(D, H, RD)[:, hs].reshape(D, HPC * RD))
        # [p, j*3+m, i, c] = wqd[256j+128i+p, 128m+c] * W8S
        wnq_sl = np.ascontiguousarray(
            wq_down[:, g * QSL:(g + 1) * QSL]).astype(np.float32) * W8S
        m["wnq"] = np.ascontiguousarray(
            wnq_sl.reshape(8, 2, 128, 3, 128)      # j, i, p, m, c
            .transpose(2, 0, 3, 1, 4)              # p, j, m, i, c
            .reshape(128, 24, 2, 128)).astype(NPF8)
        m["wkv"] = pk(wkv_down)
        m["wkuk"] = pk(wku[:, hs, :NOPE].reshape(KVR, HPC * NOPE))
        m["wkuv"] = pk(wku[:, hs, NOPE:].reshape(KVR, HPC * VD))
        m["wo"] = pk(wo.reshape(H, VD, D)[hs].reshape(HPC * VD, D))
        in_maps.append(m)
    return in_maps


LAST_EXEC_NS = None


def kernel(**inputs) -> np.ndarray:
    global LAST_EXEC_NS
    if "nc" not in _CACHED:
        _CACHED["nc"] = _build_nc()
    nc = _CACHED["nc"]
    in_maps = _host_inputs(inputs)
    trace = bool(os.environ.get("MLA_TRACE"))
    res = run_bass_kernel_spmd(nc, in_maps, core_ids=list(range(8)), trace=trace)
    LAST_EXEC_NS = res.exec_time_ns
    _CACHED["last_results"] = res
    out = np.zeros((B, S, D), np.float32)
    for core in range(8):
        b = core // HPC
        # outT is [16(mm), 4(c), 128(p), 512(s)]: d = 128*mm + p,
        # s = 512*c + s'
        ot = np.asarray(res.results[core]["outT"], dtype=np.float32)
        out[b] += ot.transpose(0, 2, 1, 3).reshape(D, S).T
    return out
